# revision 30
# baseline (speedup 1.0000x reference)
"""Bass/Trainium2 kernel for nn_AdvancedUpSampling2D (max-unpooling via scatter).

Full tensors in/out; internally sharded batch-parallel over 8 NeuronCores.

Key structural fact about the mask (argmax-style, include_batch_in_index=False):
  flat = (y * Wout + x) * C + c  with y = 2h + dy, x = 2w + dx, dy/dx in {0,1}
  Wout * C = 128 * 256 = 2^15, C = 2^8
  => dy = bit 15 of flat, dx = bit 8 of flat, and element (b,h,w,c) can only
     land at (b, 2h+dy, 2w+dx, c).  Windows are disjoint => no add-collisions.
So the scatter is a 4-way select + spatial interleave:
  out[b, 2h+dy', 2w+dx', c] = updates[b,h,w,c] * ((mask & 0x8100) == K(dy',dx'))

Optimization history (all trace-verified on HW; baseline f32 134 us):
  1. DVE compute was the real bottleneck (119 us busy, 85%): 4 fp32
     tensor_mul per chunk ran in 1x mode (2283 ns each).  Fix: all-fp16
     DVE datapath -- TT mult has a 2x_1p uop (needs ALL operands 2-byte,
     inner step 1), so indicators are produced as fp16 and the muls run
     at 2x.  (s = m & 0x8100 must stay int32: bitVec ops can't cast.)
  2. HBM traffic: storing the output f32 moved 48 MiB/core at the
     ~358 GB/s per-NC HBM limit = 140 us floor.  fp16 stores (plain
     HWDGE stores of fp16 tiles -- NOT the slow SWDGE cast-during-DMA
     store path) halve store traffic: 16 in + 16 out = 32 MiB/core.
     The host upcasts fp16->f32 during the gather; max rel err from
     fp16 rounding ~5e-4, 40x inside the 2e-2 gate.
  3. u loads via gpsimd SWDGE cast-during-DMA (f32 DRAM -> fp16 SBUF):
     removes the per-chunk DVE cast op and frees the HWDGE rings
     (112 us -> 96.6 us together with 2).
  4. 4x 16-wide chunks instead of 8x 8-wide: halves op/DMA/semaphore
     counts (per-op init, drains, and the fixed ~5 us epilogue that
     zeroes the 256-sem file scale with instruction count): -> 91.5 us.
  5. Last chunk's odd-row store on the scalar ring so the final 4 MiB
     drains on both HWDGE rings concurrently: -> 91.3 us.
Failed experiments (kept for the record): [8,16,16,16,8] uneven chunks
(Tile list-scheduler hoisted chunk j+1 indicator ops above chunk j's
last muls, +15 us); m loads issued from sync (queue behind store
sem-waits, +2.2 us); all stores on one ring with m on scalar (store
drain serialized, part of the 96.6->91.5 gap).
Final measured: 91305 ns min (fast reps 91.3-91.7; a bimodal slow mode
~104-110 appears in about half the reps regardless of schedule).
"""

import numpy as np

# Problem config (hardcoded per contract)
B, H, W, C = 16, 64, 64, 256
SY, SX = 2, 2
N_CORES = 8
BPC = B // N_CORES          # batches per core = 2
P = 128                     # partitions = BPC * H
# Uniform 16-wide chunks.  (A [8,16,16,16,8] split was tried to shrink the
# startup/tail and regressed 15us: with 5 chunks the Tile list-scheduler
# hoisted chunk j+1's indicator ops above chunk j's last two muls, pushing
# every store ~12us late.  Keep 4 uniform chunks.)
CWS = [16, 16, 16, 16]
CW = max(CWS)               # pool tiles sized for the widest chunk
NCHUNK = len(CWS)
assert sum(CWS) == W

_CACHE = {}


def _build_module():
    """Build the Bass module (single-core program, run SPMD on 8 cores)."""
    import concourse.bacc as bacc
    import concourse.tile as tile
    from concourse import mybir

    nc = bacc.Bacc(
        "TRN2",
        target_bir_lowering=False,
        debug=False,
        num_devices=N_CORES,
    )
    # Bias constants for the ScalarE activations (only 0.0/1.0 pre-registered).
    # memset on DVE, not gpsimd: the Q7 must be free to start generating the
    # SWDGE load descriptors as early as possible.
    for v in (128.0, -32896.0):
        t = nc.alloc_sbuf_tensor(f"const-float32-{v}", [128, 1], mybir.dt.float32)
        nc.vector.memset(t.ap(), v)
        nc.const_aps.aps[(mybir.dt.float32, v)] = t.ap()
    nc.all_engine_barrier()

    upd = nc.dram_tensor(
        "updates", [BPC, H, W, C], mybir.dt.float32, kind="ExternalInput"
    )
    msk = nc.dram_tensor("mask", [BPC, H, W, C], mybir.dt.int32, kind="ExternalInput")
    out = nc.dram_tensor(
        "out", [BPC, H * SY, W * SX, C], mybir.dt.float16, kind="ExternalOutput"
    )

    up_ap = upd.ap()                      # [2, 64, 64, 256]
    mk_ap = msk.ap()
    # out rows r = 2h + two  ->  [b, two, h, q, c]
    out_r = out.ap().rearrange("b (h two) q c -> b two h q c", two=SY)

    # (plane key, which row-parity tile, slot within the (w, two, c) interleave)
    PLANES = [
        (0x0000, 0, 0),  # dy=0, dx=0 -> even row, even col   (ScalarE step)
        (0x0100, 0, 1),  # dy=0, dx=1 -> even row, odd col    (DVE is_equal)
        (0x8000, 1, 0),  # dy=1, dx=0 -> odd row, even col    (DVE is_equal)
        (0x8100, 1, 1),  # dy=1, dx=1 -> odd row, odd col     (ScalarE step)
    ]

    with tile.TileContext(nc) as tc:
        PF = 2  # chunks of load prefetch ahead of compute

        with (
            tc.tile_pool(name="u16", bufs=PF + 1) as u16_pool,
            tc.tile_pool(name="m", bufs=PF + 1) as m_pool,
            tc.tile_pool(name="s", bufs=2) as s_pool,
            tc.tile_pool(name="eq", bufs=1) as eq_pool,
            tc.tile_pool(name="out", bufs=2) as out_pool,
        ):
            u_tiles, m_tiles = [], []
            W0S = [sum(CWS[:j]) for j in range(NCHUNK)]

            def emit_load(j):
                w0, cw = W0S[j], CWS[j]
                # u via SWDGE (gpsimd): casts f32->fp16 during the DMA,
                # removing the per-chunk DVE cast op entirely.  m via the
                # otherwise-idle Act HWDGE ring (RTL descriptor gen; the
                # ScalarE issue is ~0.4us and it has slack), so Q7 only
                # generates descriptors for 4 cast-loads.
                u_t = u16_pool.tile([P, cw * C], mybir.dt.float16, name="u16")
                m_t = m_pool.tile([P, cw * C], mybir.dt.int32, name="m")
                nc.gpsimd.dma_start(
                    out=u_t[:].rearrange("p (w c) -> p w c", c=C),
                    in_=up_ap[:, :, w0 : w0 + cw, :].rearrange(
                        "b h w c -> (b h) w c"
                    ),
                )
                # m loads issue from ScalarE -- except chunk 0's, which
                # issues from sync: ScalarE's head-of-program ACT_TABLE_LOADs
                # delay its first DMA issue by ~2.6us, and chunk 0's mask
                # gates ALL compute.  sync idles until the first store
                # (~22us), so one early load there is free.  (Moving ALL m
                # loads to sync regressed 2.2us: later issues queue behind
                # store sem-waits in sync's in-order program.)
                ld_m = nc.sync if j == 0 else nc.scalar
                ld_m.dma_start(
                    out=m_t[:].rearrange("p (w c) -> p w c", c=C),
                    in_=mk_ap[:, :, w0 : w0 + cw, :].rearrange("b h w c -> (b h) w c"),
                )
                u_tiles.append(u_t)
                m_tiles.append(m_t)

            # software pipeline: PF chunks of pure prefetch, then each
            # iteration issues load j+PF before chunk j's compute/stores so
            # loads never queue behind store semaphore waits, and buffer
            # reuse (bufs=PF+1) only ever waits on an already-drained chunk.
            for j in range(PF):
                emit_load(j)

            for j in range(NCHUNK):
                if j + PF < NCHUNK:
                    emit_load(j + PF)
                w0, cw = W0S[j], CWS[j]
                u_t, m_t = u_tiles[j], m_tiles[j]

                even_t = out_pool.tile([P, SX * cw * C], mybir.dt.float16, name="ev")
                odd_t = out_pool.tile([P, SX * cw * C], mybir.dt.float16, name="od")
                row_tiles = [even_t, odd_t]

                # s = m & 0x8100 (values in {0,256,32768,33024}); must stay
                # int32 -- the compiler rejects bitVec ops with output cast.
                s_t = s_pool.tile([P, cw * C], mybir.dt.int32)
                nc.vector.tensor_scalar(
                    out=s_t[:],
                    in0=m_t[:],
                    scalar1=0x8100,
                    scalar2=None,
                    op0=mybir.AluOpType.bitwise_and,
                )
                # u arrives as fp16 (cast during the SWDGE load) so the
                # muls have all-2-byte operands and hit TT's 2x_1p mode.
                # The 4 indicator planes are written into TWO column-
                # interleaved pair tiles (slot layout matches the output
                # tiles), so one broadcast TT mul per row-parity replaces
                # two muls -- 2 fewer DVE instructions per chunk (init +
                # drain + sem overhead), same element throughput.
                u_b = u_t[:].rearrange(
                    "p (w one c) -> p w one c", one=1, c=C
                ).to_broadcast([P, cw, SX, C])

                eq_pair = [
                    eq_pool.tile([P, cw * SX * C], mybir.dt.float16, name=f"eqp{par}")
                    for par in range(SY)
                ]
                for key, parity, slot in PLANES:
                    dst = eq_pair[parity][:].rearrange(
                        "p (w two c) -> p w two c", two=SX, c=C
                    )[:, :, slot, :]
                    if key == 0x0000:
                        # s==0 <=> s<128: saturated step, one ScalarE op.
                        # |arg| >= 128 always, so sigmoid returns exact 0/1.
                        nc.scalar.activation(
                            dst,
                            s_t[:].rearrange("p (w c) -> p w c", c=C),
                            mybir.ActivationFunctionType.Sigmoid,
                            bias=128.0,
                            scale=-1.0,
                        )
                    elif key == 0x8100:
                        # s==33024 <=> s>32896: saturated step, one ScalarE op
                        nc.scalar.activation(
                            dst,
                            s_t[:].rearrange("p (w c) -> p w c", c=C),
                            mybir.ActivationFunctionType.Sigmoid,
                            bias=-32896.0,
                            scale=1.0,
                        )
                    else:
                        # middle keys: exact is_equal on DVE (int32 in, fp16
                        # out -> 2x_2p; gpsimd tensor ops measured ~32us/op
                        # on HW -- never put compute there)
                        nc.vector.tensor_scalar(
                            out=dst,
                            in0=s_t[:].rearrange("p (w c) -> p w c", c=C),
                            scalar1=key,
                            scalar2=None,
                            op0=mybir.AluOpType.is_equal,
                        )
                for parity in range(SY):
                    nc.vector.tensor_mul(
                        out=row_tiles[parity][:].rearrange(
                            "p (w two c) -> p w two c", two=SX, c=C
                        ),
                        in0=u_b,
                        in1=eq_pair[parity][:].rearrange(
                            "p (w two c) -> p w two c", two=SX, c=C
                        ),
                    )

                for parity in range(SY):
                    # all stores on the SP HWDGE ring (sync does no compute,
                    # so mul-wait blocking is free) -- except the LAST
                    # chunk's odd-row store, which goes to the scalar ring
                    # so the final 4 MiB drains on two rings concurrently,
                    # halving the post-compute store tail.  (ScalarE has
                    # nothing left to do at that point.)
                    last = j == NCHUNK - 1
                    dma_eng = nc.scalar if (last and parity == 1) else nc.sync
                    dma_eng.dma_start(
                        out=out_r[:, parity, :, SX * w0 : SX * (w0 + cw), :].rearrange(
                            "b h q c -> (b h) q c"
                        ),
                        in_=row_tiles[parity][:].rearrange("p (q c) -> p q c", c=C),
                    )
    nc.finalize()
    return nc


def _get_nc():
    if "nc" not in _CACHE:
        _CACHE["nc"] = _build_module()
    return _CACHE["nc"]


def _get_runner():
    """Cached jitted shard_map executable (run_bass_via_pjrt rebuilds its jit
    closure per call, reloading the executable each time; this caches it)."""
    if "runner" in _CACHE:
        return _CACHE["runner"]
    import jax
    import jax.numpy as jnp
    from jax.experimental.shard_map import shard_map
    from jax.sharding import Mesh, PartitionSpec

    import concourse.mybir as mybir
    from concourse import bass2jax

    nc = _get_nc()
    bass2jax.install_neuronx_cc_hook()

    partition_name = nc.partition_id_tensor.name if nc.partition_id_tensor else None
    in_names, out_names, out_avals = [], [], []
    for alloc in nc.m.functions[0].allocations:
        if not isinstance(alloc, mybir.MemoryLocationSet):
            continue
        name = alloc.memorylocations[0].name
        if alloc.kind == "ExternalInput":
            if name != partition_name:
                in_names.append(name)
        elif alloc.kind == "ExternalOutput":
            out_names.append(name)
            out_avals.append(
                jax.core.ShapedArray(
                    tuple(alloc.tensor_shape), mybir.dt.np(alloc.dtype)
                )
            )
    n_params = len(in_names)
    n_outs = len(out_names)
    all_names = [*in_names, *out_names]
    if partition_name is not None:
        all_names.append(partition_name)

    def _body(*args):
        operands = list(args)
        if partition_name is not None:
            operands.append(bass2jax.partition_id_tensor())
        outs = bass2jax._bass_exec_p.bind(
            *operands,
            out_avals=tuple(out_avals),
            in_names=tuple(all_names),
            out_names=tuple(out_names),
            lowering_input_output_aliases=(),
            sim_require_finite=True,
            sim_require_nnan=True,
            nc=nc,
        )
        return tuple(outs)

    devices = jax.devices()[:N_CORES]
    mesh = Mesh(np.asarray(devices), ("core",))
    sharded = jax.jit(
        shard_map(
            _body,
            mesh=mesh,
            in_specs=(PartitionSpec("core"),) * (n_params + n_outs),
            out_specs=(PartitionSpec("core"),) * n_outs,
            check_rep=False,
        ),
        donate_argnums=tuple(range(n_params, n_params + n_outs)),
        keep_unused=True,
    )
    # Donated output buffers made on-device (no host->device zero transfer).
    zero_makers = [
        jax.jit(
            lambda shape=tuple(a.shape), dtype=a.dtype: jnp.zeros(
                (N_CORES * shape[0], *shape[1:]), dtype
            )
        )
        for a in out_avals
    ]

    def run(updates, mask):
        ins = {"updates": updates, "mask": mask}
        out_arrs = sharded(
            *[ins[name] for name in in_names], *[mk() for mk in zero_makers]
        )
        return np.asarray(out_arrs[out_names.index("out")]).astype(np.float32)

    _CACHE["runner"] = run
    return run


def _run(updates, mask, trace=False):
    updates = np.ascontiguousarray(updates, dtype=np.float32)
    mask = np.ascontiguousarray(mask, dtype=np.int32)

    if not trace:
        return _get_runner()(updates, mask), None

    # Profiling path (test.py): go through the library so NTFF capture works.
    from concourse.bass_utils import run_bass_kernel_spmd

    nc = _get_nc()
    in_maps = [
        {
            "updates": updates[i * BPC : (i + 1) * BPC],
            "mask": mask[i * BPC : (i + 1) * BPC],
        }
        for i in range(N_CORES)
    ]
    res = run_bass_kernel_spmd(
        nc,
        in_maps,
        core_ids=list(range(N_CORES)),
        trace=trace,
    )
    out = np.concatenate([r["out"] for r in res.results], axis=0).astype(np.float32)
    return out, res


def kernel(**inputs):
    out, _ = _run(inputs["updates"], inputs["mask"])
    return out

